# revision 13
# baseline (speedup 1.0000x reference)
"""Trainium2 Bass kernel for the ActionHeadGMM loss.

loss = mean_b sum_k softmax(mix)[b,k] * ( -logN(target_b | mean_bk, diag var_bk) )
with var = 5*sigmoid(cov).

Math used on device (per element, d = mean - target):
  iv  = 1/var = 0.2 + 0.2*exp(-c)         (computed as t2 + 0.2, t2 = exp(-c + ln 0.2))
  lv  = ln(1 + exp(-c)) = ln(5*t2 + 1)    (so ln var = ln 5 - lv)
  -logp[b,k] = C_k + 0.5 * sum_a (d^2*iv - lv),   C_k = 3.5*(ln 2pi + ln 5)
Since sum_k softmax = 1:
  loss = C + (0.5/B) * sum_{b,k,a} softmax(mix)[b,k] * (d^2*iv - lv)

Device computes the big sum (data-parallel over 8 cores, batch-sharded;
total reduction via ones-matmul on the idle TensorEngine, accumulated in
PSUM); host adds the constant and divides, accumulating in float64.
"""

import numpy as np

import concourse.bass as bass
import concourse.tile as tile
from concourse import bacc, mybir
from concourse.bass_utils import run_bass_kernel_spmd
from contextlib import ExitStack

P = 128          # SBUF partitions
K = 8            # mixture components
A = 7            # action dim
KA = K * A
N_CORES = 8

LN02 = float(np.log(0.2))
C_CONST = 3.5 * (float(np.log(2.0 * np.pi)) + float(np.log(5.0)))

f32 = mybir.dt.float32
bf16 = mybir.dt.bfloat16
Exp = mybir.ActivationFunctionType.Exp
Ln = mybir.ActivationFunctionType.Ln
Square = mybir.ActivationFunctionType.Square
Alu = mybir.AluOpType
AxX = mybir.AxisListType.X

# feature flags (tuned via A/B on hardware)
CFG = dict(
    pe_reduce=True,    # total sum via TensorE ones-matmul (else DVE tensor_reduce)
    d_on_pool=True,    # subtract (mean - target) on GpSimd
    d2_on_act=False,   # square on ACT (else DVE tensor_tensor)
    mixn_pool=True,    # materialize broadcast mix weights on GpSimd; dense f mult
    t2_bf16=True,      # keep exp(-c+ln.2) in bf16 (iv tensor_scalar hits 4x)
)


def build_nc(rows_per_part: int, bb: int, cfg: dict | None = None, reps: int = 1):
    cfg = {**CFG, **(cfg or {})}
    R = rows_per_part
    assert R % bb == 0
    ntiles = R // bb
    F = bb * KA          # elements/partition/tile for [b,k,a] tensors
    Fk = bb * K
    Fa = bb * A
    FC = next(c for c in range(min(F, 512), 0, -1) if F % c == 0)
    nchunks = F // FC

    nc = bacc.Bacc("TRN2", target_bir_lowering=False, debug=False)

    # activation float biases require registered const APs
    for val in (LN02,):
        t = nc.alloc_sbuf_tensor(f"const-f32-{val}", [128, 1], f32)
        nc.gpsimd.memset(t.ap(), val)
        nc.const_aps.aps[(f32, val)] = t.ap()
    nc.all_engine_barrier()

    means_d = nc.dram_tensor("means", [P, R * KA], f32, kind="ExternalInput")
    covs_d = nc.dram_tensor("covs", [P, R * KA], f32, kind="ExternalInput")
    mix_d = nc.dram_tensor("mixing", [P, R * K], f32, kind="ExternalInput")
    tg_d = nc.dram_tensor("targets", [P, R * A], f32, kind="ExternalInput")
    if cfg["pe_reduce"]:
        out_d = nc.dram_tensor("out", [1, FC], f32, kind="ExternalOutput")
    else:
        out_d = nc.dram_tensor("out", [P, ntiles], f32, kind="ExternalOutput")

    t2dt = bf16 if cfg["t2_bf16"] else f32

    with tile.TileContext(nc) as tc, ExitStack() as exs:
        io = exs.enter_context(tc.tile_pool(name="io", bufs=3))
        mid = exs.enter_context(tc.tile_pool(name="mid", bufs=2))
        accp = exs.enter_context(tc.tile_pool(name="accp", bufs=1))

        if cfg["pe_reduce"]:
            psp = exs.enter_context(tc.tile_pool(name="psum", bufs=1, space="PSUM"))
            psum_full = psp.tile([P, FC], f32)
            psum = psum_full[0:1, :]
            ones = accp.tile([P, 1], bf16)
            nc.gpsimd.memset(ones[:, :], 1.0)
        else:
            acc = accp.tile([P, ntiles], f32)

        for rep in range(reps):
          for t in range(ntiles):
            m_t = io.tile([P, F], f32, tag="m")
            c_t = io.tile([P, F], f32, tag="c")
            mx_t = io.tile([P, Fk], f32, tag="mx")
            tg_t = io.tile([P, Fa], f32, tag="tg")
            nc.sync.dma_start(out=m_t[:, :], in_=means_d[:, t * F:(t + 1) * F])
            nc.sync.dma_start(out=c_t[:, :], in_=covs_d[:, t * F:(t + 1) * F])
            nc.sync.dma_start(out=mx_t[:, :], in_=mix_d[:, t * Fk:(t + 1) * Fk])
            nc.sync.dma_start(out=tg_t[:, :], in_=tg_d[:, t * Fa:(t + 1) * Fa])

            # covariance side: t2 = 0.2*exp(-c); lv = ln(1+5*t2); iv = t2+0.2
            t2_t = mid.tile([P, F], t2dt, tag="t2")
            lv_t = mid.tile([P, F], bf16, tag="lv")
            iv_t = mid.tile([P, F], bf16, tag="iv")
            nc.scalar.activation(t2_t[:, :], c_t[:, :], Exp, bias=LN02, scale=-1.0)
            nc.scalar.activation(lv_t[:, :], t2_t[:, :], Ln, bias=1.0, scale=5.0)
            nc.vector.tensor_scalar(iv_t[:, :], t2_t[:, :], 0.2, None, Alu.add)

            # d = mean - target (broadcast target over k)
            d_t = mid.tile([P, F], bf16, tag="d")
            m_v = m_t[:, :].rearrange("p (b k a) -> p b k a", b=bb, k=K, a=A)
            tg_v = (
                tg_t[:, :]
                .rearrange("p (b a) -> p b a", b=bb, a=A)
                .unsqueeze(2)
                .broadcast_to([P, bb, K, A])
            )
            d_v = d_t[:, :].rearrange("p (b k a) -> p b k a", b=bb, k=K, a=A)
            d_eng = nc.gpsimd if cfg["d_on_pool"] else nc.vector
            d_eng.tensor_tensor(d_v, m_v, tg_v, Alu.subtract)

            # q = d^2 * iv ; e = q - lv
            d2_t = mid.tile([P, F], bf16, tag="d2")
            if cfg["d2_on_act"]:
                nc.scalar.activation(d2_t[:, :], d_t[:, :], Square)
            else:
                nc.vector.tensor_tensor(d2_t[:, :], d_t[:, :], d_t[:, :], Alu.mult)
            q_t = mid.tile([P, F], bf16, tag="q")
            nc.vector.tensor_tensor(q_t[:, :], d2_t[:, :], iv_t[:, :], Alu.mult)
            e_t = mid.tile([P, F], bf16, tag="e")
            nc.vector.tensor_tensor(e_t[:, :], q_t[:, :], lv_t[:, :], Alu.subtract)

            # softmax over k: mixn = exp(mx) / sum_k exp(mx)
            em_t = mid.tile([P, Fk], f32, tag="em")
            nc.scalar.activation(em_t[:, :], mx_t[:, :], Exp)
            s_t = mid.tile([P, bb], f32, tag="s")
            em_v = em_t[:, :].rearrange("p (b k) -> p b k", b=bb, k=K)
            nc.vector.reduce_sum(s_t[:, :], em_v, AxX)
            r_t = mid.tile([P, bb], f32, tag="r")
            nc.vector.reciprocal(r_t[:, :], s_t[:, :])
            mixn_t = mid.tile([P, Fk], f32, tag="mixn")
            mixn_v = mixn_t[:, :].rearrange("p (b k) -> p b k", b=bb, k=K)
            r_v = r_t[:, :].unsqueeze(2).broadcast_to([P, bb, K])
            nc.vector.tensor_tensor(mixn_v, em_v, r_v, Alu.mult)

            # f = e * mixn (broadcast over a)
            f_t = mid.tile([P, F], bf16, tag="f")
            mixn_b = (
                mixn_t[:, :]
                .rearrange("p (b k) -> p b k", b=bb, k=K)
                .unsqueeze(3)
                .broadcast_to([P, bb, K, A])
            )
            if cfg["mixn_pool"]:
                mf_t = mid.tile([P, F], bf16, tag="mf")
                mf_v = mf_t[:, :].rearrange("p (b k a) -> p b k a", b=bb, k=K, a=A)
                nc.gpsimd.tensor_copy(mf_v, mixn_b)
                nc.vector.tensor_tensor(f_t[:, :], e_t[:, :], mf_t[:, :], Alu.mult)
            else:
                f_v = f_t[:, :].rearrange("p (b k a) -> p b k a", b=bb, k=K, a=A)
                e_v = e_t[:, :].rearrange("p (b k a) -> p b k a", b=bb, k=K, a=A)
                nc.vector.tensor_tensor(f_v, e_v, mixn_b, Alu.mult)

            # total-sum reduction
            if cfg["pe_reduce"]:
                for ci in range(nchunks):
                    nc.tensor.matmul(
                        psum[:, :],
                        ones[:, :],
                        f_t[:, ci * FC:(ci + 1) * FC],
                        start=(rep == 0 and t == 0 and ci == 0),
                        stop=(rep == reps - 1 and t == ntiles - 1
                              and ci == nchunks - 1),
                    )
            else:
                nc.vector.tensor_reduce(acc[:, t:t + 1], f_t[:, :], AxX, Alu.add)

        if cfg["pe_reduce"]:
            osb = accp.tile([1, FC], f32)
            nc.vector.tensor_copy(osb[:, :], psum[:, :])
            nc.sync.dma_start(out=out_d[:, :], in_=osb[:, :])
        else:
            nc.sync.dma_start(out=out_d[:, :], in_=acc[:, :])

    nc.compile()
    return nc


_NC_CACHE: dict = {}


def _get_nc(rows_per_part: int, bb: int):
    key = (rows_per_part, bb)
    if key not in _NC_CACHE:
        _NC_CACHE[key] = build_nc(rows_per_part, bb)
    return _NC_CACHE[key]


def make_in_maps(means, covariances, mixing_coefficients, action_targets):
    B = means.shape[0]
    Bc = B // N_CORES
    R = Bc // P
    in_maps = []
    for c in range(N_CORES):
        sl = slice(c * Bc, (c + 1) * Bc)
        in_maps.append({
            "means": np.ascontiguousarray(
                means[sl], dtype=np.float32).reshape(P, R * KA),
            "covs": np.ascontiguousarray(
                covariances[sl], dtype=np.float32).reshape(P, R * KA),
            "mixing": np.ascontiguousarray(
                mixing_coefficients[sl], dtype=np.float32).reshape(P, R * K),
            "targets": np.ascontiguousarray(
                action_targets[sl], dtype=np.float32).reshape(P, R * A),
        })
    return in_maps


def kernel(means, covariances, mixing_coefficients, action_targets):
    B = means.shape[0]
    Bc = B // N_CORES
    R = Bc // P
    bb = 32 if R % 32 == 0 else (8 if R % 8 == 0 else 1)
    nc = _get_nc(R, bb)
    in_maps = make_in_maps(means, covariances, mixing_coefficients, action_targets)
    res = run_bass_kernel_spmd(nc, in_maps, core_ids=list(range(N_CORES)))
    total = sum(
        np.asarray(r["out"]).astype(np.float64).sum() for r in res.results
    )
    loss = C_CONST + 0.5 * total / B
    return np.float32(loss)
